# revision 40
# baseline (speedup 1.0000x reference)
"""AVRoPE (axial video RoPE + audio token) Trainium2 Bass kernel.

Problem (hardcoded shapes):
  x_video: (4, 16, 4096, 96) f32   [B, H, NF*P*P, D] with NF=P=16, D=96
  x_audio: (4, 16, 16, 96)   f32
  cos/sin: (16, 17, 17, 48)  f32
  offset:  scalar int (always 0 here since cos first dim == NF)

Reference semantics (per (b,h) pair, fully elementwise):
  video token t=(n,y,x), y,x<16:  x0=x[0::2], x1=x[1::2]
     y0 = x0*c(n,y,x) - x1*s(n,y,x);  y1 = x1*c + x0*s;  out = [y0, y1]
  audio token n: same with c(n,16,16), s(n,16,16).

Sharding: data-parallel over flattened (B*H)=64 -> 8 (b,h) pairs per core.
cos/sin replicated; no cross-core comms.

Device layout per core:
  xv  (32768, 96) viewed as 8 tiles of (128, 3072): partition j of tile i
      holds tokens [i*4096 + j*32, ... +32). Each tile covers one (b,h)'s
      4096 tokens, so one resident cos/sin tile pair serves all 8 tiles.
  cos/sin shipped 48-wide (4096, 48); the h (half) axis is broadcast
  on-chip with a zero-stride AP dim. Per tile:
    y(k,h,c)  = x(k, 2c+h) * c(k, c)        # [x0*c | x1*c]
    q(k,h,c)  = x(k, 2c+h) * s(k, c)        # [x0*s | x1*s]
    y(k,0,c) -= q(k,1,c)                    # y0 = x0*c - x1*s
    y(k,1,c) += q(k,0,c)                    # y1 = x1*c + x0*s
  Audio: one (128, 96) tile per core (rows = bh*16 + n), same 4 ops.
"""

import numpy as np

B, H, NF, P, D = 4, 16, 16, 16, 96
BH = B * H
N_CORES = 8
BH_PER_CORE = BH // N_CORES          # 8
LV = NF * P * P                      # 4096 video tokens per (b,h)
HALF = D // 2                        # 48
K = 32                               # tokens per partition per video tile
FD = K * D                           # 3072 free-dim elements per partition
FDH = K * HALF                       # 1536 (const tile width)
VTILES = BH_PER_CORE * LV // (128 * K)   # 8

_NC_CACHE = {}


def _build_nc(adds="vector"):
    """adds: 'vector' (best measured: inputs on SP HWDGE ring, consts +
    outputs on ACT HWDGE ring, 4 DVE tensor_tensor ops per tile) |
    'v3' (chunked head/tail, two-ring spread) | 'gpsimd' | 'alt' | 'acc' |
    'pe' (TensorE identity-matmul adds)."""
    import concourse.bass as bass
    import concourse.bacc as bacc
    import concourse.mybir as mybir
    from concourse.tile import TileContext
    from contextlib import ExitStack

    f32 = mybir.dt.float32
    mult = mybir.AluOpType.mult
    add = mybir.AluOpType.add
    sub = mybir.AluOpType.subtract

    nc = bacc.Bacc("TRN2", target_bir_lowering=False, debug=False)
    xv = nc.declare_dram_parameter("xv", [BH_PER_CORE * LV, D], f32, isOutput=False)
    xa = nc.declare_dram_parameter("xa", [128, D], f32, isOutput=False)
    cv = nc.declare_dram_parameter("cv", [LV, HALF], f32, isOutput=False)
    sv = nc.declare_dram_parameter("sv", [LV, HALF], f32, isOutput=False)
    svn = nc.declare_dram_parameter("svn", [LV, HALF], f32, isOutput=False)
    ca = nc.declare_dram_parameter("ca", [128, HALF], f32, isOutput=False)
    sa = nc.declare_dram_parameter("sa", [128, HALF], f32, isOutput=False)
    yv = nc.declare_dram_parameter("yv", [BH_PER_CORE * LV, D], f32, isOutput=True)
    ya = nc.declare_dram_parameter("ya", [128, D], f32, isOutput=True)
    if adds == "pe":
        idn = nc.declare_dram_parameter("ident", [128, 128], f32, isOutput=False)

    xv_t = xv.rearrange("(t p k) d -> t p (k d)", t=VTILES, p=128, k=K)
    yv_t = yv.rearrange("(t p k) d -> t p (k d)", t=VTILES, p=128, k=K)
    xv4 = xv.rearrange("(t p k) d -> t p k d", t=VTILES, p=128, k=K)
    yv4 = yv.rearrange("(t p k) d -> t p k d", t=VTILES, p=128, k=K)
    cv_t = cv.rearrange("(p k) d -> p (k d)", p=128, k=K)
    sv_t = sv.rearrange("(p k) d -> p (k d)", p=128, k=K)
    cv3 = cv.rearrange("(p k) d -> p k d", p=128, k=K)
    sv3 = sv.rearrange("(p k) d -> p k d", p=128, k=K)
    svn_t = svn.rearrange("(p k) d -> p (k d)", p=128, k=K)

    with TileContext(nc) as tc, ExitStack() as ctx:
        const = ctx.enter_context(tc.tile_pool(name="const", bufs=1))
        cvt = const.tile([128, FDH], f32, tag="cvt")
        svt = const.tile([128, FDH], f32, tag="svt")
        cat = const.tile([128, HALF], f32, tag="cat")
        sat = const.tile([128, HALF], f32, tag="sat")
        # consts go on the ACT HWDGE ring so they don't serialize with the
        # x-tile loads on the SP ring. Quartered + interleaved so the first
        # chunk of compute unblocks as early as possible.
        NQ = 4
        KQ = K // NQ
        if adds in ("v3", "pe"):
            for q in range(NQ):
                ks = slice(q * KQ * HALF, (q + 1) * KQ * HALF)
                nc.scalar.dma_start(cvt[:, ks], cv3[:, q * KQ:(q + 1) * KQ, :])
                nc.scalar.dma_start(svt[:, ks], sv3[:, q * KQ:(q + 1) * KQ, :])
            nc.scalar.dma_start(cat[:], ca[:, :])
            nc.scalar.dma_start(sat[:], sa[:, :])
            if adds == "pe":
                identt = const.tile([128, 128], f32, tag="id")
                nc.scalar.dma_start(identt[:], idn[:, :])
        else:
            nc.scalar.dma_start(cat[:], ca[:, :])
            nc.scalar.dma_start(sat[:], sa[:, :])
            nc.scalar.dma_start(cvt[:], cv_t)
            nc.scalar.dma_start(svt[:], sv_t)


        xp = ctx.enter_context(tc.tile_pool(name="x", bufs=5 if adds == "v3" else 3))
        yp = ctx.enter_context(tc.tile_pool(name="y", bufs=3))
        qp = ctx.enter_context(tc.tile_pool(name="q", bufs=3))

        def stt_acc(out, in0, sgn, in1):
            # out += (in0 * sgn) * in1   via ScalarTensorTensor + AddAccumulate
            return nc.vector.add_instruction(mybir.InstTensorScalarPtr(
                name=nc.get_next_instruction_name(),
                is_scalar_tensor_tensor=True,
                op0=mult, op1=mult,
                acc="AddAccumulate",
                ins=[nc.vector.lower_ap(in0),
                     nc.vector.lower_ap_or_imm(float(sgn)),
                     nc.vector.lower_ap(in1)],
                outs=[nc.vector.lower_ap(out)],
            ))

        def rope_acc_ops(xt, yt, ct, st, k):
            x_de = xt[:].rearrange("p (k c h) -> p k h c", k=k, c=HALF, h=2)
            y4 = yt[:].rearrange("p (k h c) -> p k h c", k=k, h=2, c=HALF)
            s3 = st.rearrange("p (k c) -> p k c", k=k, c=HALF)
            c4 = _bcast(ct, k)
            nc.vector.tensor_tensor(y4, x_de, c4, mult)
            stt_acc(y4[:, :, 0, :], x_de[:, :, 1, :], -1.0, s3)  # y0 -= x1*s
            stt_acc(y4[:, :, 1, :], x_de[:, :, 0, :], 1.0, s3)   # y1 += x0*s

        def rope_ops(xt, yt, qt, ct, st, k, add_engine):
            # x in interleaved order: idx(k, h, c) = k*96 + 2c + h
            x_de = xt[:].rearrange("p (k c h) -> p k h c", k=k, c=HALF, h=2)
            # y / q in halves order: idx(k, h, c) = k*96 + 48h + c
            y4 = yt[:].rearrange("p (k h c) -> p k h c", k=k, h=2, c=HALF)
            q4 = qt[:].rearrange("p (k h c) -> p k h c", k=k, h=2, c=HALF)
            c4 = _bcast(ct, k)
            s4 = _bcast(st, k)
            nc.vector.tensor_tensor(y4, x_de, c4, mult)
            nc.vector.tensor_tensor(q4, x_de, s4, mult)
            add_engine.tensor_tensor(y4[:, :, 0, :], y4[:, :, 0, :], q4[:, :, 1, :], sub)
            add_engine.tensor_tensor(y4[:, :, 1, :], y4[:, :, 1, :], q4[:, :, 0, :], add)

        def _bcast(ct, k):
            v = ct.rearrange("p (k c) -> p k c", k=k, c=HALF)
            import dataclasses
            # insert a [0, 2] AP dim between k and c
            ap = v.ap.copy()
            ap.insert(2, [0, 2])
            return dataclasses.replace(v, ap=ap)

        # audio first: tiny, overlaps the big const/x0 DMAs. Output goes on
        # the SWDGE (gpsimd) stream so it never blocks HWDGE input issue.
        xat = xp.tile([128, D], f32, tag="xa")
        nc.sync.dma_start(xat[:], xa[:, :])
        yat = yp.tile([128, D], f32, tag="ya")
        qat = qp.tile([128, D], f32, tag="qa")
        rope_ops(xat, yat, qat, cat[:], sat[:], 1, nc.vector)
        if adds != "v3":
            nc.scalar.dma_start(ya[:, :], yat[:])

        def rope_region(xt, yt, qt, k0, kn):
            # compute on tokens [k0, k0+kn) per partition of a K-wide tile
            x_de = xt[:].rearrange("p (k c h) -> p k h c", k=K, c=HALF, h=2)[:, k0:k0 + kn]
            y4 = yt[:].rearrange("p (k h c) -> p k h c", k=K, h=2, c=HALF)[:, k0:k0 + kn]
            q4 = qt[:].rearrange("p (k h c) -> p k h c", k=K, h=2, c=HALF)[:, k0:k0 + kn]
            c4 = _bcast(cvt[:], K)[:, k0:k0 + kn]
            s4 = _bcast(svt[:], K)[:, k0:k0 + kn]
            nc.vector.tensor_tensor(y4, x_de, c4, mult)
            nc.vector.tensor_tensor(q4, x_de, s4, mult)
            nc.vector.tensor_tensor(y4[:, :, 0, :], y4[:, :, 0, :], q4[:, :, 1, :], sub)
            nc.vector.tensor_tensor(y4[:, :, 1, :], y4[:, :, 1, :], q4[:, :, 0, :], add)

        if adds == "pe":
            pp = ctx.enter_context(tc.tile_pool(name="ps", bufs=2, space="PSUM"))
            KH = K // 2
            FDH2 = KH * D                      # 1536

            # First matmul-accumulate into a fresh PSUM bank ignores prior
            # DVE writes (stale bank state after NEFF load). Warm every bank
            # of both slots with a start=True matmul during the idle head.
            for _w in range(2):
                ptw = pp.tile([128, FDH2], f32, tag="ps")
                for b in range(FDH2 // 512):
                    nc.tensor.matmul(
                        ptw[:, b * 512:(b + 1) * 512], identt[:],
                        cvt[:, 0:512], start=True, stop=True,
                        skip_group_check=True)

            def rope_pe_half(xt, yt, qt, k0):
                # products on DVE; add on PE (identity matmul accumulate
                # into PSUM); PSUM->SBUF copy on ACT
                x_de = xt[:].rearrange("p (k c h) -> p k h c",
                                       k=K, c=HALF, h=2)[:, k0:k0 + KH]
                q4 = qt[:].rearrange("p (k h c) -> p k h c",
                                     k=K, h=2, c=HALF)[:, k0:k0 + KH]
                s3 = svt[:].rearrange("p (k c) -> p k c",
                                      k=K, c=HALF)[:, k0:k0 + KH]
                c4 = _bcast(cvt[:], K)[:, k0:k0 + KH]
                pt = pp.tile([128, FDH2], f32, tag="ps")
                p4 = pt[:].rearrange("p (k h c) -> p k h c", k=KH, h=2, c=HALF)
                nc.vector.tensor_tensor(p4, x_de, c4, mult)
                nc.vector.scalar_tensor_tensor(
                    q4[:, :, 0, :], x_de[:, :, 1, :], -1.0, s3, mult, mult)
                nc.vector.scalar_tensor_tensor(
                    q4[:, :, 1, :], x_de[:, :, 0, :], 1.0, s3, mult, mult)
                for b in range(FDH2 // 512):
                    nc.tensor.matmul(
                        pt[:, b * 512:(b + 1) * 512], identt[:],
                        qt[:, k0 * D + b * 512:k0 * D + (b + 1) * 512],
                        start=False, stop=True, skip_group_check=True)
                nc.scalar.activation(
                    yt[:, k0 * D:(k0 + KH) * D], pt[:],
                    mybir.ActivationFunctionType.Copy)

            in_eng = {0: nc.sync, 1: nc.scalar, 2: nc.sync, 3: nc.scalar,
                      4: nc.sync, 5: nc.scalar, 6: nc.sync, 7: nc.scalar}
            tail_out = [nc.gpsimd, nc.sync, nc.scalar, nc.gpsimd]
            for i in range(VTILES):
                xt = xp.tile([128, FD], f32, tag="x")
                yt = yp.tile([128, FD], f32, tag="y")
                qt = qp.tile([128, FD], f32, tag="q")
                head = i == 0
                tail = i == VTILES - 1
                in_chunks = NQ if head else 1
                for q in range(in_chunks):
                    kq = K // in_chunks
                    in_eng[i].dma_start(
                        xt[:, q * kq * D:(q + 1) * kq * D],
                        xv4[i, :, q * kq:(q + 1) * kq, :])
                rope_pe_half(xt, yt, qt, 0)
                rope_pe_half(xt, yt, qt, KH)
                if tail:
                    for q in range(NQ):
                        kq = K // NQ
                        tail_out[q].dma_start(
                            yv4[i, :, q * kq:(q + 1) * kq, :],
                            yt[:, q * kq * D:(q + 1) * kq * D])
                else:
                    nc.gpsimd.dma_start(yv_t[i], yt[:])
        elif adds == "v3":
            # Two HWDGE rings only (no SWDGE: Q7 descriptor generation drags
            # the DVE via the shared SBUF port). Inputs interleaved across
            # both rings and all emitted before any output so input issue is
            # never blocked behind an output's data dependency.
            in_eng = {0: nc.sync, 1: nc.scalar, 2: nc.sync, 3: nc.scalar,
                      4: nc.sync, 5: nc.scalar, 6: nc.sync, 7: nc.scalar}
            out_eng = {0: nc.scalar, 1: nc.sync, 2: nc.scalar, 3: nc.sync,
                       4: nc.scalar, 5: nc.sync, 6: nc.scalar, 7: None}
            tail_out = [nc.sync, nc.scalar, nc.sync, nc.scalar]
            xts = []
            for i in range(VTILES):
                xt = xp.tile([128, FD], f32, tag="x")
                xts.append(xt)
                in_chunks = NQ if i == 0 else 1
                for q in range(in_chunks):
                    kq = K // in_chunks
                    fs = slice(q * kq * D, (q + 1) * kq * D)
                    in_eng[i].dma_start(xt[:, fs], xv4[i, :, q * kq:(q + 1) * kq, :])
            for i in range(VTILES):
                xt = xts[i]
                yt = yp.tile([128, FD], f32, tag="y")
                qt = qp.tile([128, FD], f32, tag="q")
                tail = i == VTILES - 1
                cmp_chunks = NQ if i == 0 else 1
                out_chunks = NQ if tail else 1
                for q in range(cmp_chunks):
                    kq = K // cmp_chunks
                    rope_region(xt, yt, qt, q * kq, kq)
                for q in range(out_chunks):
                    kq = K // out_chunks
                    fs = slice(q * kq * D, (q + 1) * kq * D)
                    eng = tail_out[q] if tail else out_eng[i]
                    eng.dma_start(yv4[i, :, q * kq:(q + 1) * kq, :], yt[:, fs])
                if i == 1:
                    # audio out: data ready since the head; emitted here so
                    # it never blocks input issue on the scalar stream
                    nc.scalar.dma_start(ya[:, :], yat[:])
        else:
            for i in range(VTILES):
                if adds == "vector":
                    add_eng = nc.vector
                elif adds == "gpsimd":
                    add_eng = nc.gpsimd
                else:
                    add_eng = nc.gpsimd if (i % 2 == 0) else nc.vector
                xt = xp.tile([128, FD], f32, tag="x")
                nc.sync.dma_start(xt[:], xv_t[i])
                yt = yp.tile([128, FD], f32, tag="y")
                if adds == "acc":
                    rope_acc_ops(xt, yt, cvt[:], svt[:], K)
                else:
                    qt = qp.tile([128, FD], f32, tag="q")
                    rope_ops(xt, yt, qt, cvt[:], svt[:], K, add_eng)
                if i == VTILES - 1:
                    # tail: sync ring is idle once inputs are done; split the
                    # last store across both HWDGE rings to drain faster
                    kh = K // 2
                    nc.scalar.dma_start(yv4[i, :, :kh, :], yt[:, :kh * D])
                    nc.sync.dma_start(yv4[i, :, kh:, :], yt[:, kh * D:])
                else:
                    nc.scalar.dma_start(yv_t[i], yt[:])

    nc.finalize()
    return nc


def _get_nc(**kw):
    key = tuple(sorted(kw.items()))
    if key not in _NC_CACHE:
        _NC_CACHE[key] = _build_nc(**kw)
    return _NC_CACHE[key]


def _prep_consts(cos, sin, offset):
    c = np.asarray(cos, np.float32)[offset:offset + NF]   # (16,17,17,48)
    s = np.asarray(sin, np.float32)[offset:offset + NF]
    cv = np.ascontiguousarray(c[:, :P, :P, :]).reshape(LV, HALF)
    sv = np.ascontiguousarray(s[:, :P, :P, :]).reshape(LV, HALF)
    ca = np.ascontiguousarray(np.tile(c[:, P, P, :], (BH_PER_CORE, 1)))  # (128,48)
    sa = np.ascontiguousarray(np.tile(s[:, P, P, :], (BH_PER_CORE, 1)))
    return cv, sv, ca, sa, -sv


def _make_in_maps(x_video, x_audio, cos, sin, offset):
    x_video = np.ascontiguousarray(np.asarray(x_video), dtype=np.float32)
    x_audio = np.ascontiguousarray(np.asarray(x_audio), dtype=np.float32)
    off = int(np.asarray(offset))
    cv, sv, ca, sa, svn = _prep_consts(cos, sin, off)
    xvf = x_video.reshape(BH, LV, D)
    xaf = x_audio.reshape(BH, NF, D)
    in_maps = []
    for c0 in range(N_CORES):
        sl = slice(c0 * BH_PER_CORE, (c0 + 1) * BH_PER_CORE)
        in_maps.append({
            "xv": xvf[sl].reshape(BH_PER_CORE * LV, D),
            "xa": xaf[sl].reshape(BH_PER_CORE * NF, D),
            "cv": cv, "sv": sv, "ca": ca, "sa": sa, "svn": svn,
            "ident": np.eye(128, dtype=np.float32),
        })
    return in_maps


def _gather(results):
    yv = np.stack([np.asarray(r["yv"]).reshape(BH_PER_CORE, LV, D)
                   for r in results]).reshape(B, H, LV, D)
    ya = np.stack([np.asarray(r["ya"]).reshape(BH_PER_CORE, NF, D)
                   for r in results]).reshape(B, H, NF, D)
    return yv, ya


def _expected_inputs(nc):
    import concourse.mybir as mybir
    names = set()
    for alloc in nc.m.functions[0].allocations:
        if isinstance(alloc, mybir.MemoryLocationSet) and alloc.kind == "ExternalInput":
            names.add(alloc.memorylocations[0].name)
    return names


def run(x_video, x_audio, cos, sin, offset, trace=False, **nc_kw):
    from concourse.bass_utils import run_bass_kernel_spmd
    nc = _get_nc(**nc_kw)
    want = _expected_inputs(nc)
    in_maps = [{k: v for k, v in m.items() if k in want}
               for m in _make_in_maps(x_video, x_audio, cos, sin, offset)]
    res = run_bass_kernel_spmd(nc, in_maps, list(range(N_CORES)), trace=trace)
    yv, ya = _gather(res.results)
    return (yv, ya), res


def kernel(x_video, x_audio, cos, sin, offset):
    (yv, ya), _ = run(x_video, x_audio, cos, sin, offset, trace=False)
    return yv, ya


# revision 41
# speedup vs baseline: 1.0271x; 1.0271x over previous
"""AVRoPE (axial video RoPE + audio token) Trainium2 Bass kernel.

Problem (hardcoded shapes):
  x_video: (4, 16, 4096, 96) f32   [B, H, NF*P*P, D] with NF=P=16, D=96
  x_audio: (4, 16, 16, 96)   f32
  cos/sin: (16, 17, 17, 48)  f32
  offset:  scalar int (always 0 here since cos first dim == NF)

Reference semantics (per (b,h) pair, fully elementwise):
  video token t=(n,y,x), y,x<16:  x0=x[0::2], x1=x[1::2]
     y0 = x0*c(n,y,x) - x1*s(n,y,x);  y1 = x1*c + x0*s;  out = [y0, y1]
  audio token n: same with c(n,16,16), s(n,16,16).

Sharding: data-parallel over flattened (B*H)=64 -> 8 (b,h) pairs per core.
cos/sin replicated; no cross-core comms.

Device layout per core:
  xv  (32768, 96) viewed as 8 tiles of (128, 3072): partition j of tile i
      holds tokens [i*4096 + j*32, ... +32). Each tile covers one (b,h)'s
      4096 tokens, so one resident cos/sin tile pair serves all 8 tiles.
  cos/sin shipped 48-wide (4096, 48); the h (half) axis is broadcast
  on-chip with a zero-stride AP dim. Per tile:
    y(k,h,c)  = x(k, 2c+h) * c(k, c)        # [x0*c | x1*c]
    q(k,h,c)  = x(k, 2c+h) * s(k, c)        # [x0*s | x1*s]
    y(k,0,c) -= q(k,1,c)                    # y0 = x0*c - x1*s
    y(k,1,c) += q(k,0,c)                    # y1 = x1*c + x0*s
  Audio: one (128, 96) tile per core (rows = bh*16 + n), same 4 ops.
"""

import numpy as np

B, H, NF, P, D = 4, 16, 16, 16, 96
BH = B * H
N_CORES = 8
BH_PER_CORE = BH // N_CORES          # 8
LV = NF * P * P                      # 4096 video tokens per (b,h)
HALF = D // 2                        # 48
K = 32                               # tokens per partition per video tile
FD = K * D                           # 3072 free-dim elements per partition
FDH = K * HALF                       # 1536 (const tile width)
VTILES = BH_PER_CORE * LV // (128 * K)   # 8

_NC_CACHE = {}


def _build_nc(adds="vector"):
    """adds: 'vector' (best measured: inputs on SP HWDGE ring, consts +
    outputs on ACT HWDGE ring, 4 DVE tensor_tensor ops per tile) |
    'v3' (chunked head/tail, two-ring spread) | 'gpsimd' | 'alt' | 'acc' |
    'pe' (TensorE identity-matmul adds)."""
    import concourse.bass as bass
    import concourse.bacc as bacc
    import concourse.mybir as mybir
    from concourse.tile import TileContext
    from contextlib import ExitStack

    f32 = mybir.dt.float32
    mult = mybir.AluOpType.mult
    add = mybir.AluOpType.add
    sub = mybir.AluOpType.subtract

    nc = bacc.Bacc("TRN2", target_bir_lowering=False, debug=False)
    xv = nc.declare_dram_parameter("xv", [BH_PER_CORE * LV, D], f32, isOutput=False)
    xa = nc.declare_dram_parameter("xa", [128, D], f32, isOutput=False)
    cv = nc.declare_dram_parameter("cv", [LV, HALF], f32, isOutput=False)
    sv = nc.declare_dram_parameter("sv", [LV, HALF], f32, isOutput=False)
    svn = nc.declare_dram_parameter("svn", [LV, HALF], f32, isOutput=False)
    ca = nc.declare_dram_parameter("ca", [128, HALF], f32, isOutput=False)
    sa = nc.declare_dram_parameter("sa", [128, HALF], f32, isOutput=False)
    yv = nc.declare_dram_parameter("yv", [BH_PER_CORE * LV, D], f32, isOutput=True)
    ya = nc.declare_dram_parameter("ya", [128, D], f32, isOutput=True)
    if adds == "pe":
        idn = nc.declare_dram_parameter("ident", [128, 128], f32, isOutput=False)

    xv_t = xv.rearrange("(t p k) d -> t p (k d)", t=VTILES, p=128, k=K)
    yv_t = yv.rearrange("(t p k) d -> t p (k d)", t=VTILES, p=128, k=K)
    xv4 = xv.rearrange("(t p k) d -> t p k d", t=VTILES, p=128, k=K)
    yv4 = yv.rearrange("(t p k) d -> t p k d", t=VTILES, p=128, k=K)
    cv_t = cv.rearrange("(p k) d -> p (k d)", p=128, k=K)
    sv_t = sv.rearrange("(p k) d -> p (k d)", p=128, k=K)
    cv3 = cv.rearrange("(p k) d -> p k d", p=128, k=K)
    sv3 = sv.rearrange("(p k) d -> p k d", p=128, k=K)
    svn_t = svn.rearrange("(p k) d -> p (k d)", p=128, k=K)

    with TileContext(nc) as tc, ExitStack() as ctx:
        const = ctx.enter_context(tc.tile_pool(name="const", bufs=1))
        cvt = const.tile([128, FDH], f32, tag="cvt")
        svt = const.tile([128, FDH], f32, tag="svt")
        cat = const.tile([128, HALF], f32, tag="cat")
        sat = const.tile([128, HALF], f32, tag="sat")
        # consts go on the ACT HWDGE ring so they don't serialize with the
        # x-tile loads on the SP ring. Quartered + interleaved so the first
        # chunk of compute unblocks as early as possible.
        NQ = 4
        KQ = K // NQ
        if adds in ("v3", "pe"):
            for q in range(NQ):
                ks = slice(q * KQ * HALF, (q + 1) * KQ * HALF)
                nc.scalar.dma_start(cvt[:, ks], cv3[:, q * KQ:(q + 1) * KQ, :])
                nc.scalar.dma_start(svt[:, ks], sv3[:, q * KQ:(q + 1) * KQ, :])
            nc.scalar.dma_start(cat[:], ca[:, :])
            nc.scalar.dma_start(sat[:], sa[:, :])
            if adds == "pe":
                identt = const.tile([128, 128], f32, tag="id")
                nc.scalar.dma_start(identt[:], idn[:, :])
        else:
            nc.scalar.dma_start(cat[:], ca[:, :])
            nc.scalar.dma_start(sat[:], sa[:, :])
            nc.scalar.dma_start(cvt[:], cv_t)
            nc.scalar.dma_start(svt[:], sv_t)


        xp = ctx.enter_context(tc.tile_pool(name="x", bufs=5 if adds == "v3" else 3))
        yp = ctx.enter_context(tc.tile_pool(name="y", bufs=3))
        qp = ctx.enter_context(tc.tile_pool(name="q", bufs=3))

        def stt_acc(out, in0, sgn, in1):
            # out += (in0 * sgn) * in1   via ScalarTensorTensor + AddAccumulate
            return nc.vector.add_instruction(mybir.InstTensorScalarPtr(
                name=nc.get_next_instruction_name(),
                is_scalar_tensor_tensor=True,
                op0=mult, op1=mult,
                acc="AddAccumulate",
                ins=[nc.vector.lower_ap(in0),
                     nc.vector.lower_ap_or_imm(float(sgn)),
                     nc.vector.lower_ap(in1)],
                outs=[nc.vector.lower_ap(out)],
            ))

        def rope_acc_ops(xt, yt, ct, st, k):
            x_de = xt[:].rearrange("p (k c h) -> p k h c", k=k, c=HALF, h=2)
            y4 = yt[:].rearrange("p (k h c) -> p k h c", k=k, h=2, c=HALF)
            s3 = st.rearrange("p (k c) -> p k c", k=k, c=HALF)
            c4 = _bcast(ct, k)
            nc.vector.tensor_tensor(y4, x_de, c4, mult)
            stt_acc(y4[:, :, 0, :], x_de[:, :, 1, :], -1.0, s3)  # y0 -= x1*s
            stt_acc(y4[:, :, 1, :], x_de[:, :, 0, :], 1.0, s3)   # y1 += x0*s

        def rope_ops(xt, yt, qt, ct, st, k, add_engine):
            # x in interleaved order: idx(k, h, c) = k*96 + 2c + h
            x_de = xt[:].rearrange("p (k c h) -> p k h c", k=k, c=HALF, h=2)
            # y / q in halves order: idx(k, h, c) = k*96 + 48h + c
            y4 = yt[:].rearrange("p (k h c) -> p k h c", k=k, h=2, c=HALF)
            q4 = qt[:].rearrange("p (k h c) -> p k h c", k=k, h=2, c=HALF)
            c4 = _bcast(ct, k)
            s4 = _bcast(st, k)
            nc.vector.tensor_tensor(y4, x_de, c4, mult)
            nc.vector.tensor_tensor(q4, x_de, s4, mult)
            add_engine.tensor_tensor(y4[:, :, 0, :], y4[:, :, 0, :], q4[:, :, 1, :], sub)
            add_engine.tensor_tensor(y4[:, :, 1, :], y4[:, :, 1, :], q4[:, :, 0, :], add)

        def _bcast(ct, k):
            v = ct.rearrange("p (k c) -> p k c", k=k, c=HALF)
            import dataclasses
            # insert a [0, 2] AP dim between k and c
            ap = v.ap.copy()
            ap.insert(2, [0, 2])
            return dataclasses.replace(v, ap=ap)

        # audio first: tiny, overlaps the big const/x0 DMAs. Output goes on
        # the SWDGE (gpsimd) stream so it never blocks HWDGE input issue.
        xat = xp.tile([128, D], f32, tag="xa")
        nc.sync.dma_start(xat[:], xa[:, :])
        yat = yp.tile([128, D], f32, tag="ya")
        qat = qp.tile([128, D], f32, tag="qa")
        rope_ops(xat, yat, qat, cat[:], sat[:], 1, nc.vector)
        if adds != "v3":
            nc.scalar.dma_start(ya[:, :], yat[:])

        def rope_region(xt, yt, qt, k0, kn):
            # compute on tokens [k0, k0+kn) per partition of a K-wide tile
            x_de = xt[:].rearrange("p (k c h) -> p k h c", k=K, c=HALF, h=2)[:, k0:k0 + kn]
            y4 = yt[:].rearrange("p (k h c) -> p k h c", k=K, h=2, c=HALF)[:, k0:k0 + kn]
            q4 = qt[:].rearrange("p (k h c) -> p k h c", k=K, h=2, c=HALF)[:, k0:k0 + kn]
            c4 = _bcast(cvt[:], K)[:, k0:k0 + kn]
            s4 = _bcast(svt[:], K)[:, k0:k0 + kn]
            nc.vector.tensor_tensor(y4, x_de, c4, mult)
            nc.vector.tensor_tensor(q4, x_de, s4, mult)
            nc.vector.tensor_tensor(y4[:, :, 0, :], y4[:, :, 0, :], q4[:, :, 1, :], sub)
            nc.vector.tensor_tensor(y4[:, :, 1, :], y4[:, :, 1, :], q4[:, :, 0, :], add)

        if adds == "pe":
            pp = ctx.enter_context(tc.tile_pool(name="ps", bufs=2, space="PSUM"))
            KH = K // 2
            FDH2 = KH * D                      # 1536

            # First matmul-accumulate into a fresh PSUM bank ignores prior
            # DVE writes (stale bank state after NEFF load). Warm every bank
            # of both slots with a start=True matmul during the idle head.
            for _w in range(2):
                ptw = pp.tile([128, FDH2], f32, tag="ps")
                for b in range(FDH2 // 512):
                    nc.tensor.matmul(
                        ptw[:, b * 512:(b + 1) * 512], identt[:],
                        cvt[:, 0:512], start=True, stop=True,
                        skip_group_check=True)

            def rope_pe_half(xt, yt, qt, k0):
                # products on DVE; add on PE (identity matmul accumulate
                # into PSUM); PSUM->SBUF copy on ACT
                x_de = xt[:].rearrange("p (k c h) -> p k h c",
                                       k=K, c=HALF, h=2)[:, k0:k0 + KH]
                q4 = qt[:].rearrange("p (k h c) -> p k h c",
                                     k=K, h=2, c=HALF)[:, k0:k0 + KH]
                s3 = svt[:].rearrange("p (k c) -> p k c",
                                      k=K, c=HALF)[:, k0:k0 + KH]
                c4 = _bcast(cvt[:], K)[:, k0:k0 + KH]
                pt = pp.tile([128, FDH2], f32, tag="ps")
                p4 = pt[:].rearrange("p (k h c) -> p k h c", k=KH, h=2, c=HALF)
                nc.vector.tensor_tensor(p4, x_de, c4, mult)
                nc.vector.scalar_tensor_tensor(
                    q4[:, :, 0, :], x_de[:, :, 1, :], -1.0, s3, mult, mult)
                nc.vector.scalar_tensor_tensor(
                    q4[:, :, 1, :], x_de[:, :, 0, :], 1.0, s3, mult, mult)
                for b in range(FDH2 // 512):
                    nc.tensor.matmul(
                        pt[:, b * 512:(b + 1) * 512], identt[:],
                        qt[:, k0 * D + b * 512:k0 * D + (b + 1) * 512],
                        start=False, stop=True, skip_group_check=True)
                nc.scalar.activation(
                    yt[:, k0 * D:(k0 + KH) * D], pt[:],
                    mybir.ActivationFunctionType.Copy)

            in_eng = {0: nc.sync, 1: nc.scalar, 2: nc.sync, 3: nc.scalar,
                      4: nc.sync, 5: nc.scalar, 6: nc.sync, 7: nc.scalar}
            tail_out = [nc.gpsimd, nc.sync, nc.scalar, nc.gpsimd]
            for i in range(VTILES):
                xt = xp.tile([128, FD], f32, tag="x")
                yt = yp.tile([128, FD], f32, tag="y")
                qt = qp.tile([128, FD], f32, tag="q")
                head = i == 0
                tail = i == VTILES - 1
                in_chunks = NQ if head else 1
                for q in range(in_chunks):
                    kq = K // in_chunks
                    in_eng[i].dma_start(
                        xt[:, q * kq * D:(q + 1) * kq * D],
                        xv4[i, :, q * kq:(q + 1) * kq, :])
                rope_pe_half(xt, yt, qt, 0)
                rope_pe_half(xt, yt, qt, KH)
                if tail:
                    for q in range(NQ):
                        kq = K // NQ
                        tail_out[q].dma_start(
                            yv4[i, :, q * kq:(q + 1) * kq, :],
                            yt[:, q * kq * D:(q + 1) * kq * D])
                else:
                    nc.gpsimd.dma_start(yv_t[i], yt[:])
        elif adds == "v3":
            # Two HWDGE rings only (no SWDGE: Q7 descriptor generation drags
            # the DVE via the shared SBUF port). Inputs interleaved across
            # both rings and all emitted before any output so input issue is
            # never blocked behind an output's data dependency.
            in_eng = {0: nc.sync, 1: nc.scalar, 2: nc.sync, 3: nc.scalar,
                      4: nc.sync, 5: nc.scalar, 6: nc.sync, 7: nc.scalar}
            out_eng = {0: nc.scalar, 1: nc.sync, 2: nc.scalar, 3: nc.sync,
                       4: nc.scalar, 5: nc.sync, 6: nc.scalar, 7: None}
            tail_out = [nc.sync, nc.scalar, nc.sync, nc.scalar]
            xts = []
            for i in range(VTILES):
                xt = xp.tile([128, FD], f32, tag="x")
                xts.append(xt)
                in_chunks = NQ if i == 0 else 1
                for q in range(in_chunks):
                    kq = K // in_chunks
                    fs = slice(q * kq * D, (q + 1) * kq * D)
                    in_eng[i].dma_start(xt[:, fs], xv4[i, :, q * kq:(q + 1) * kq, :])
            for i in range(VTILES):
                xt = xts[i]
                yt = yp.tile([128, FD], f32, tag="y")
                qt = qp.tile([128, FD], f32, tag="q")
                tail = i == VTILES - 1
                cmp_chunks = NQ if i == 0 else 1
                out_chunks = NQ if tail else 1
                for q in range(cmp_chunks):
                    kq = K // cmp_chunks
                    rope_region(xt, yt, qt, q * kq, kq)
                for q in range(out_chunks):
                    kq = K // out_chunks
                    fs = slice(q * kq * D, (q + 1) * kq * D)
                    eng = tail_out[q] if tail else out_eng[i]
                    eng.dma_start(yv4[i, :, q * kq:(q + 1) * kq, :], yt[:, fs])
                if i == 1:
                    # audio out: data ready since the head; emitted here so
                    # it never blocks input issue on the scalar stream
                    nc.scalar.dma_start(ya[:, :], yat[:])
        else:
            for i in range(VTILES):
                if adds == "vector":
                    add_eng = nc.vector
                elif adds == "gpsimd":
                    add_eng = nc.gpsimd
                else:
                    add_eng = nc.gpsimd if (i % 2 == 0) else nc.vector
                xt = xp.tile([128, FD], f32, tag="x")
                nc.sync.dma_start(xt[:], xv_t[i])
                yt = yp.tile([128, FD], f32, tag="y")
                if adds == "acc":
                    rope_acc_ops(xt, yt, cvt[:], svt[:], K)
                else:
                    qt = qp.tile([128, FD], f32, tag="q")
                    rope_ops(xt, yt, qt, cvt[:], svt[:], K, add_eng)
                nc.scalar.dma_start(yv_t[i], yt[:])

    nc.finalize()
    return nc


def _get_nc(**kw):
    key = tuple(sorted(kw.items()))
    if key not in _NC_CACHE:
        _NC_CACHE[key] = _build_nc(**kw)
    return _NC_CACHE[key]


def _prep_consts(cos, sin, offset):
    c = np.asarray(cos, np.float32)[offset:offset + NF]   # (16,17,17,48)
    s = np.asarray(sin, np.float32)[offset:offset + NF]
    cv = np.ascontiguousarray(c[:, :P, :P, :]).reshape(LV, HALF)
    sv = np.ascontiguousarray(s[:, :P, :P, :]).reshape(LV, HALF)
    ca = np.ascontiguousarray(np.tile(c[:, P, P, :], (BH_PER_CORE, 1)))  # (128,48)
    sa = np.ascontiguousarray(np.tile(s[:, P, P, :], (BH_PER_CORE, 1)))
    return cv, sv, ca, sa, -sv


def _make_in_maps(x_video, x_audio, cos, sin, offset):
    x_video = np.ascontiguousarray(np.asarray(x_video), dtype=np.float32)
    x_audio = np.ascontiguousarray(np.asarray(x_audio), dtype=np.float32)
    off = int(np.asarray(offset))
    cv, sv, ca, sa, svn = _prep_consts(cos, sin, off)
    xvf = x_video.reshape(BH, LV, D)
    xaf = x_audio.reshape(BH, NF, D)
    in_maps = []
    for c0 in range(N_CORES):
        sl = slice(c0 * BH_PER_CORE, (c0 + 1) * BH_PER_CORE)
        in_maps.append({
            "xv": xvf[sl].reshape(BH_PER_CORE * LV, D),
            "xa": xaf[sl].reshape(BH_PER_CORE * NF, D),
            "cv": cv, "sv": sv, "ca": ca, "sa": sa, "svn": svn,
            "ident": np.eye(128, dtype=np.float32),
        })
    return in_maps


def _gather(results):
    yv = np.stack([np.asarray(r["yv"]).reshape(BH_PER_CORE, LV, D)
                   for r in results]).reshape(B, H, LV, D)
    ya = np.stack([np.asarray(r["ya"]).reshape(BH_PER_CORE, NF, D)
                   for r in results]).reshape(B, H, NF, D)
    return yv, ya


def _expected_inputs(nc):
    import concourse.mybir as mybir
    names = set()
    for alloc in nc.m.functions[0].allocations:
        if isinstance(alloc, mybir.MemoryLocationSet) and alloc.kind == "ExternalInput":
            names.add(alloc.memorylocations[0].name)
    return names


def run(x_video, x_audio, cos, sin, offset, trace=False, **nc_kw):
    from concourse.bass_utils import run_bass_kernel_spmd
    nc = _get_nc(**nc_kw)
    want = _expected_inputs(nc)
    in_maps = [{k: v for k, v in m.items() if k in want}
               for m in _make_in_maps(x_video, x_audio, cos, sin, offset)]
    res = run_bass_kernel_spmd(nc, in_maps, list(range(N_CORES)), trace=trace)
    yv, ya = _gather(res.results)
    return (yv, ya), res


def kernel(x_video, x_audio, cos, sin, offset):
    (yv, ya), _ = run(x_video, x_audio, cos, sin, offset, trace=False)
    return yv, ya
